# revision 44
# baseline (speedup 1.0000x reference)
"""Trainium2 Bass kernel for nn_Attention_45406394253435 (gnn segment attention).

Full-input contract: kernel(**inputs) takes the unsharded numpy inputs and
returns the full [N, C] output. Internally shards across 8 NeuronCores at
segment boundaries (batch is sorted), runs a Bass/Tile kernel per core, and
gathers.

Math (per point i in segment b):
    qp    = q @ Wq.T  (+ bq)
    attn  = qp * kp[b] / sqrt(DH);  e = exp(attn)
    s[b]  = sum_{i in b} e[i]
    out   = (e * vp[b]/s[b]) @ Wo.T + bo
Key identities used:
  * The exp bias bq*kp[b]/sqrt(DH) is constant per (segment, channel) and
    cancels exactly in the segment softmax -> dropped on device.
  * kp[b]/sqrt(DH) is folded into per-segment fp8 GEMM1 weights (scaled by
    512 to sit in fp8e4m3 normal range; descaled by a constant 1/512 in the
    exp activation). GEMM1 runs fp8 DoubleRow (K=256 in one pass).
  * Segment softmax max-subtraction cancels mathematically; attn is O(5) so
    exp is safe in f32.
Per-core pipeline (slot = one segment):
    DMA q (fp8) -> GEMM1 (fp8 DoubleRow, per-slot weights) -> exp+accum (ACT,
    1024-col chunks, bf16 out) -> stats (DVE: reduce/sub/recip/fused wp)
    -> GEMM2 (bf16, folded Wo) -> PSUM->SBUF copy+bias (DVE) -> DMA out (bf16)
"""

import math

import numpy as np

N = 131072
B = 64
C = 256
H = 8
DH = C // H
NCORES = 8
SEGS_PER_CORE = B // NCORES  # 8 slots per core
NB = C // 128  # channel partition blocks (2)
WQ_SCALE = 512.0  # fp8 range scaling for kbs-folded GEMM1 weights


def _mk_chunks(sp, cw=1024):
    """Split a slot into PSUM-tile chunks: full `cw`s plus one tail."""
    gs, off = [], 0
    while off < sp:
        w = min(cw, sp - off)
        gs.append((off, w))
        off += w
    return gs


def _mk_chunks1(sp):
    """GEMM1/exp chunks (same 1024-wide chunking as GEMM2)."""
    return _mk_chunks(sp)


def _build_bass(slot_pads, repeats=1, timing_io=False, unroll=1):
    """Build the per-core program. Executes `repeats * unroll` bodies:
    a For_i hardware loop over `repeats` (all-engine barrier per iteration)
    around `unroll` back-to-back software-pipelined bodies."""
    import contextlib

    import concourse.bacc as bacc
    import concourse.mybir as mybir
    import concourse.tile as tile

    f32 = mybir.dt.float32
    bf16 = mybir.dt.bfloat16
    fp8 = mybir.dt.float8e4

    slot_pads = tuple(slot_pads)
    NP = sum(slot_pads)
    offs = [0]
    for sp in slot_pads:
        offs.append(offs[-1] + sp)
    max_pad = max(slot_pads)
    slot_chunks = [_mk_chunks(sp) for sp in slot_pads]
    slot_chunks1 = [_mk_chunks1(sp) for sp in slot_pads]
    NGMAX = max(len(g) for g in slot_chunks1)

    nc = bacc.Bacc("TRN2", target_bir_lowering=False, debug=False,
                   num_devices=NCORES)

    qT_cols = max_pad if timing_io else NP
    qT_d = nc.dram_tensor("qT", [C, qT_cols], fp8, kind="ExternalInput").ap()
    wq8_d = nc.dram_tensor("wq8", [128, SEGS_PER_CORE * NB * C], fp8,
                           kind="ExternalInput").ap()
    vp_d = nc.dram_tensor("vp", [128, NB * SEGS_PER_CORE], f32,
                          kind="ExternalInput").ap()
    corr_d = nc.dram_tensor("corr", [128, NB * SEGS_PER_CORE], f32,
                            kind="ExternalInput").ap()
    wot_d = nc.dram_tensor("wot", [C, C], bf16, kind="ExternalInput").ap()
    bo_d = nc.dram_tensor("bo_b", [128, NB], f32, kind="ExternalInput").ap()
    out_cols = max_pad if timing_io else NP
    out_d = nc.dram_tensor("out", [C, out_cols], bf16,
                           kind="ExternalOutput").ap()

    with tile.TileContext(nc) as tc:
        with (
            tc.tile_pool(name="const", bufs=1) as cpool,
            tc.tile_pool(name="qp", bufs=4) as qpool,
            tc.tile_pool(name="ep", bufs=4) as epool,
            tc.tile_pool(name="sp", bufs=2) as spool,
            tc.tile_pool(name="wp", bufs=2) as wpool,
            tc.tile_pool(name="op", bufs=3) as opool,
            tc.tile_pool(name="ps1", bufs=2, space="PSUM") as ps1,
            tc.tile_pool(name="ps2", bufs=2, space="PSUM") as ps2,
        ):
            # constants
            wq8_t = cpool.tile([128, SEGS_PER_CORE, NB, C], fp8, tag="wq8")
            nc.sync.dma_start(
                wq8_t[:],
                wq8_d[:].rearrange("p (j t m) -> p j t m",
                                   j=SEGS_PER_CORE, t=NB))
            vp_t = cpool.tile([128, NB * SEGS_PER_CORE], f32, tag="vp")
            nc.sync.dma_start(vp_t[:], vp_d[:])
            corr_t = cpool.tile([128, NB * SEGS_PER_CORE], f32, tag="corr")
            nc.sync.dma_start(corr_t[:], corr_d[:])
            wot_t = []
            for cb in range(NB):
                t = cpool.tile([128, C], bf16, tag=f"wot{cb}")
                nc.sync.dma_start(t[:], wot_d[cb * 128:(cb + 1) * 128, :])
                wot_t.append(t)
            bo_t = cpool.tile([128, NB], f32, tag="bo")
            nc.sync.dma_start(bo_t[:], bo_d[:])

            def body():
                _emit_body(nc, tc, mybir, slot_pads, offs, slot_chunks,
                           slot_chunks1, NGMAX,
                           qpool, epool, spool, wpool, opool, ps1, ps2,
                           qT_d, out_d, wq8_t, wot_t, vp_t, corr_t, bo_t,
                           timing_io)

            rep_ctx = (tc.For_i(0, repeats, 1) if repeats > 1
                       else contextlib.nullcontext())
            with rep_ctx:
                for _ in range(unroll):
                    body()

    nc.compile()
    return nc


def _emit_body(nc, tc, mybir, slot_pads, offs, slot_chunks, slot_chunks1,
               NGMAX,
               qpool, epool, spool, wpool, opool, ps1, ps2,
               qT_d, out_d, wq8_t, wot_t, vp_t, corr_t, bo_t,
               timing_io=False):
    f32 = mybir.dt.float32
    bf16 = mybir.dt.bfloat16
    fp8 = mybir.dt.float8e4
    Exp = mybir.ActivationFunctionType.Exp
    X = mybir.AxisListType.X
    DR = mybir.MatmulPerfMode.DoubleRow

    # Slot-level software pipeline, cb-major GEMM1 within a slot (loose
    # PSUM coupling), phase2(j-1) interleaved with phase1(j) at step level.
    # PSUM->SBUF copies run on DVE, except every ACT_COPY_EVERY-th which
    # runs on ACT to balance engine busy time.
    ACT_COPY_EVERY = 10 ** 9
    SEGS = len(slot_pads)
    copy_idx = [0]

    def phase1_steps(j):
        sp = slot_pads[j]
        base = 0 if timing_io else offs[j]
        qm = qpool.tile([128, NB, sp], fp8, tag="q", name=f"q_{j}")
        e_t = [epool.tile([128, sp], bf16, tag=f"e{cb}",
                          name=f"e{cb}_{j}") for cb in range(NB)]
        s_parts = spool.tile([128, NB, NGMAX + 1], f32, tag="spart",
                             name=f"spart_{j}")
        ng = len(slot_chunks1[j])
        steps = []

        def dma_in():
            nc.sync.dma_start(
                qm[:],
                qT_d[:, base:base + sp]
                .rearrange("(b p) w -> p b w", p=128))
            # park -corr in accumulator column ng so the stats reduce yields
            # the corrected segment sum directly (saves one serial DVE hop)
            nc.vector.tensor_scalar_mul(
                s_parts[:, :, ng:ng + 1],
                corr_t[:, NB * j:NB * (j + 1)]
                .rearrange("p (c o) -> p c o", o=1),
                -1.0)
        steps.append(dma_in)

        def mk_cb(cb):
            # one step per cb: all chunks' matmuls share the same stationary
            # operand back-to-back on PE (minimizes DoubleRow weight reloads)
            def emit():
                wq_sl = wq8_t[:, j, :, cb * 128:(cb + 1) * 128]
                for g, (off, w) in enumerate(slot_chunks1[j]):
                    p = ps1.tile([128, 2, 512], f32, tag="p",
                                 name=f"p_{j}_{cb}_{g}")
                    pf = p.rearrange("p a b -> p (a b)")
                    for h0 in range(0, w, 512):
                        hw = min(512, w - h0)
                        nc.tensor.matmul(
                            pf[:, h0:h0 + hw],
                            wq_sl,
                            qm[:, :, off + h0:off + h0 + hw],
                            start=True, stop=True, perf_mode=DR)
                    nc.scalar.activation(
                        e_t[cb][:, off:off + w], pf[:, 0:w], Exp,
                        scale=float(1.0 / WQ_SCALE),
                        accum_out=s_parts[:, cb, g:g + 1])
            return emit
        for cb in range(NB):
            steps.append(mk_cb(cb))
        return steps, (j, e_t, s_parts)

    def phase2_steps(state, jpos):
        j, e_t, s_parts = state
        ng = len(slot_chunks1[j])
        wp_t = [wpool.tile([128, C], bf16, tag=f"wp{cb}", name=f"wp{cb}_{j}")
                for cb in range(NB)]
        steps = []

        def stats():
            # s = sum(parts + [-corr]);  wp = WoT * (1/s) * vp   (DVE)
            s_val = spool.tile([128, NB], f32, tag="sval", name=f"sval_{j}")
            nc.vector.reduce_sum(s_val[:], s_parts[:, :, 0:ng + 1], axis=X)
            r_t = spool.tile([128, NB], f32, tag="rt", name=f"rt_{j}")
            nc.vector.reciprocal(r_t[:], s_val[:])
            mult = mybir.AluOpType.mult
            for cb in range(NB):
                nc.vector.tensor_scalar(
                    wp_t[cb][:], wot_t[cb][:],
                    r_t[:, cb:cb + 1],
                    vp_t[:, NB * j + cb:NB * j + cb + 1],
                    op0=mult, op1=mult)
        steps.append(stats)

        sp = slot_pads[j]
        out_stage = opool.tile([128, NB, sp], bf16, tag="ostage",
                               name=f"ostage_{j}")

        def mk_chunk(g, off, w):
            def emit():
                for cbp in range(NB):
                    po = ps2.tile([128, 2, 512], f32, tag="po",
                                  name=f"po_{j}_{g}_{cbp}")
                    pof = po.rearrange("p a b -> p (a b)")
                    for h0 in range(0, w, 512):
                        hw = min(512, w - h0)
                        for kb in range(NB):
                            nc.tensor.matmul(
                                pof[:, h0:h0 + hw],
                                wp_t[kb][:, cbp * 128:(cbp + 1) * 128],
                                e_t[kb][:, off + h0:off + h0 + hw],
                                start=(kb == 0), stop=(kb == NB - 1))
                    copy_idx[0] += 1
                    if copy_idx[0] % ACT_COPY_EVERY == 6:
                        nc.scalar.activation(
                            out_stage[:, cbp, off:off + w], pof[:, 0:w],
                            mybir.ActivationFunctionType.Identity,
                            bias=bo_t[:, cbp:cbp + 1])
                    else:
                        nc.vector.tensor_scalar_add(
                            out_stage[:, cbp, off:off + w], pof[:, 0:w],
                            bo_t[:, cbp:cbp + 1])
            return emit
        for g, (off, w) in enumerate(slot_chunks[j]):
            steps.append(mk_chunk(g, off, w))

        obase = 0 if timing_io else offs[j]

        def dma_out():
            nc.gpsimd.dma_start(
                out_d[:, obase:obase + sp]
                .rearrange("(b p) w -> p b w", p=128),
                out_stage[:])
        steps.append(dma_out)
        return steps

    def interleave(a, b, delay=0):
        out = list(a[:delay])
        n = max(len(a) - delay, len(b))
        for i in range(n):
            if delay + i < len(a):
                out.append(a[delay + i])
            if i < len(b):
                out.append(b[i])
        return out

    prev = None
    for jpos in range(SEGS):
        s1, state = phase1_steps(jpos)
        s2 = phase2_steps(prev, jpos - 1) if prev is not None else []
        for step in interleave(s1, s2):
            step()
        prev = state
    for step in phase2_steps(prev, SEGS - 1):
        step()


def _plan(batch):
    counts = np.bincount(np.asarray(batch).astype(np.int64), minlength=B)
    starts = np.concatenate([[0], np.cumsum(counts)])
    order = np.argsort(-counts, kind="stable")
    assign = [[int(order[SEGS_PER_CORE * j + c]) for j in range(SEGS_PER_CORE)]
              for c in range(NCORES)]
    slot_pads = tuple(
        max(256, int(-(-int(counts[order[SEGS_PER_CORE * j:
                                         SEGS_PER_CORE * (j + 1)]].max())
                       // 64) * 64))
        for j in range(SEGS_PER_CORE))
    offs = [0]
    for sp in slot_pads:
        offs.append(offs[-1] + sp)
    return counts, starts, assign, slot_pads, offs


def _host_prep(q, k, v, batch, Wq, bq, Wk, bk, Wv, bv, Wo, bo, plan):
    import ml_dtypes

    f = np.float32
    bf = ml_dtypes.bfloat16
    f8 = ml_dtypes.float8_e4m3
    counts, starts, assign, slot_pads, offs = plan
    q8 = np.asarray(q, dtype=f).astype(f8)
    kp = (np.asarray(k, f) @ np.asarray(Wk, f).T + np.asarray(bk, f))
    vp = (np.asarray(v, f) @ np.asarray(Wv, f).T + np.asarray(bv, f))
    kbs = kp / f(math.sqrt(DH))                     # [B, C]
    NP = offs[-1]
    Wq = np.asarray(Wq, f)

    in_maps = []
    wot = np.ascontiguousarray(np.asarray(Wo, f).T.astype(bf))
    bo_b = np.ascontiguousarray(np.asarray(bo, f).reshape(NB, 128).T)
    for c in range(NCORES):
        qT = np.zeros((C, NP), dtype=f8)
        vp_c = np.empty((128, NB * SEGS_PER_CORE), dtype=f)
        corr_c = np.empty((128, NB * SEGS_PER_CORE), dtype=f)
        segs = [assign[c][j] for j in range(SEGS_PER_CORE)]
        # per-slot GEMM1 weights: Wq[m, k] * kbs[b, m] * WQ_SCALE, laid out
        # as [p=k%128, j, t=k//128, m]
        wq_f = (Wq[None, :, :] *
                (kbs[segs] * f(WQ_SCALE))[:, :, None])   # [j, m, k]
        wq8 = np.ascontiguousarray(
            wq_f.reshape(SEGS_PER_CORE, C, NB, 128)
            .transpose(3, 0, 2, 1).astype(f8))           # [p, j, t, m]
        for j in range(SEGS_PER_CORE):
            b = segs[j]
            n = counts[b]
            qT[:, offs[j]:offs[j] + n] = q8[starts[b]:starts[b + 1]].T
            for cb in range(NB):
                sl = slice(cb * 128, (cb + 1) * 128)
                vp_c[:, NB * j + cb] = vp[b][sl]
                corr_c[:, NB * j + cb] = f(slot_pads[j] - n)
        in_maps.append({
            "qT": qT, "vp": vp_c, "corr": corr_c,
            "wq8": wq8.reshape(128, SEGS_PER_CORE * NB * C),
            "wot": wot, "bo_b": bo_b,
        })
    return in_maps


def _gather(results, plan):
    counts, starts, assign, slot_pads, offs = plan
    out = np.empty((N, C), dtype=np.float32)
    for c in range(NCORES):
        o = np.asarray(results[c]["out"]).astype(np.float32)
        for j in range(SEGS_PER_CORE):
            b = assign[c][j]
            n = counts[b]
            out[starts[b]:starts[b + 1]] = o[:, offs[j]:offs[j] + n].T
    return out


_CACHE = {}


def _get_bass(slot_pads):
    if slot_pads not in _CACHE:
        _CACHE[slot_pads] = _build_bass(slot_pads)
    return _CACHE[slot_pads]


def kernel(q, k, v, batch, Wq, bq, Wk, bk, Wv, bv, Wo, bo):
    import concourse.bass_utils as bass_utils

    plan = _plan(batch)
    in_maps = _host_prep(q, k, v, batch, Wq, bq, Wk, bk, Wv, bv, Wo, bo, plan)
    nc = _get_bass(plan[3])

    last_err = None
    for attempt in range(3):  # device exec is rarely flaky; retry
        try:
            res = bass_utils.run_bass_kernel_spmd(
                nc, in_maps, core_ids=list(range(NCORES)))
            return _gather(res.results, plan)
        except Exception as e:  # noqa: BLE001
            last_err = e
            # Drop cached executables and give the device time to
            # self-recover before retrying in-process.
            import time

            try:
                import jax

                jax.clear_caches()
            except Exception:  # noqa: BLE001
                pass
            time.sleep(5 * (attempt + 1))
    raise last_err
